# revision 33
# baseline (speedup 1.0000x reference)
"""Trainium2 Bass kernel for nn_CAM (channel attention module), fp8 edition.

Reference computation (per batch element n):
    v = x[n].reshape(C, H*W)                      # [512, 4096]
    energy = v @ v.T                              # [512, 512]
    attn = softmax(energy, axis=-1)
    out = attn @ v                                # [512, 4096]
    result = para_mu * out + x[n]

Sharding: data-parallel over batch N=8, one batch element per NeuronCore
(8 cores). Everything is core-local — no collectives.

Numerics: for unit-gaussian x, energy's diagonal (||v_i||^2 ~ 4096) towers
~3500 above the off-diagonal (max ~450), so softmax is within float eps of
one-hot and fp8e4m3 quantization of v / exp leaves the result well inside
tolerance (2.1e-3 max-rel measured on hardware against a 2e-2 gate).

Kernel strategy (per core), ~71 us measured vs the 110 us f32r baseline:
  1. Input lands in ONE stacked f32r tile [128, MT, HW] via 11 DMAs (7
     512-col slabs + the last slab at k-tile grain for a short tail chain).
  2. ACT/DVE casts each slab to fp8e4 into V8 [128, MT, HW] (stacked
     channel tiles share partitions — the layout DoubleRow's paired-k-tile
     operand slicing needs). Pool/GPSIMD is deliberately unused: it is a
     software (Q7) engine on this stack and measured far below the other
     engines on hardware.
  3. PE transposes fp8 v-slabs into vts [128, KT, C]. Walrus requires fp8
     transpose outputs at element step 2, so the PSUM staging tile carries
     a trailing stride-2 dim; PSUM->SBUF copies compact it, one paired copy
     per double-k-tile (per-k at the tail), alternating DVE/ACT.
     (A uint16-paired variant — half the transposes, 2x-rate copies —
     simmed faster but its byte-strided DoubleRow operands hard-crashed
     the exec unit; per-k stride-2 middle-dim slices are what hardware
     accepts.)
  4. Energy in fp8e4 DoubleRow (0.5 cyc/row, K=256 per instruction), full
     512-wide rows: softmax reads E straight from PSUM, no reconstruction.
  5. Row softmax: DVE reduce_max (negated) from PSUM -> ACT Exp with
     per-partition bias, fp32 row-sum accumulation BEFORE the fp8 cast of
     the exp rows (normalization stays exact; the drain scale carries
     pm/rowsum). The Exp table is preloaded during the load phase. Row
     tile 0's chain is emitted first on every queue; later rows' stats
     interleave into phase 4 so they never delay the first output bytes.
  6. expT via fp8 PE transposes; out_unnorm = expT.T @ V8 in DoubleRow,
     tt-outer per half-row so each [128, 2, 128] weight load amortizes
     over 4 x 512 moving columns. Accumulators rotate over 6 PSUM banks
     (2 o-banks + the 4 energy banks, dead after the exp pass).
  7. Residual fold: a third matmul per chunk accumulates diag(rs/pm) @ x
     (f32r, 1 cyc/row) into the same PSUM bank, so the epilogue is a
     single per-chunk PSUM drain with the pm/rowsum scale (ACT and DVE
     alternating) — no separate elementwise add pass anywhere.
  8. Out ships on SP: quarter-row 512 KB pieces for the first and last row
     tiles (single-chunk first pieces), 1 MB halves in between.
"""

import sys

if "/opt/trn_rl_repo" not in sys.path:
    sys.path.insert(0, "/opt/trn_rl_repo")

from contextlib import ExitStack

import numpy as np

import concourse.bass as bass
import concourse.mybir as mybir
import concourse.tile as tile
from concourse import bacc
from concourse.bass_utils import run_bass_kernel_spmd
from concourse.masks import make_identity

N, C, H, W = 8, 512, 64, 64
HW = H * W            # 4096
P = 128               # partitions
MT = C // P           # 4 row tiles of the channel dim
KT = HW // P          # 32 contraction k-tiles for the energy matmul
TT = KT // 2          # 16 DoubleRow double-k-tiles
NCH = 512             # free-dim chunk for the output matmul (one PSUM bank)
NCHUNKS = HW // NCH   # 8
LCH = HW // 8         # 512-column DMA load slabs
F32 = mybir.dt.float32
F32R = mybir.dt.float32r
F8 = mybir.dt.float8e4
DR = mybir.MatmulPerfMode.DoubleRow


def _body(ctx: ExitStack, tc: "tile.TileContext", out: bass.AP, x: bass.AP, pm: bass.AP,
          reps: int = 1):
    nc = tc.nc
    consts = ctx.enter_context(tc.tile_pool(name="consts", bufs=1))
    v_pool = ctx.enter_context(tc.tile_pool(name="v", bufs=1))
    v8_pool = ctx.enter_context(tc.tile_pool(name="v8", bufs=1))
    vt_pool = ctx.enter_context(tc.tile_pool(name="vt", bufs=1))
    exp_pool = ctx.enter_context(tc.tile_pool(name="exp", bufs=1))
    expt_pool = ctx.enter_context(tc.tile_pool(name="expt", bufs=1))
    stat_pool = ctx.enter_context(tc.tile_pool(name="stats", bufs=1))
    out_pool = ctx.enter_context(tc.tile_pool(name="ob", bufs=2))
    e_psum = ctx.enter_context(tc.tile_pool(name="e_ps", bufs=1, space="PSUM"))
    t_psum = ctx.enter_context(tc.tile_pool(name="t_ps", bufs=2, space="PSUM"))
    o_psum = ctx.enter_context(tc.tile_pool(name="o_ps", bufs=2, space="PSUM"))

    identity = consts.tile([P, P], F32)
    nc.vector.memset(identity, 0.0)
    make_identity(nc, identity, nomemset=True)
    # fp8 twin: transposes of fp8 data need an fp8 permutation ifmap (and run
    # at 1.0 cyc/row). 0.0/1.0 are exact in fp8e4.
    identity8 = consts.tile([P, P], F8)
    nc.vector.tensor_copy(out=identity8, in_=identity)

    # emitted after make_identity: the gpsimd queue is serial, and this DMA
    # ahead of affine_select would delay the Pool cast stream
    pm_tile = consts.tile([P, 1], F32)
    nc.gpsimd.dma_start(out=pm_tile, in_=pm.to_broadcast((P, 1)))
    # Preload the Exp activation table (1283 ns) while DMAs stream so the
    # post-barrier softmax doesn't eat the table switch.
    exp_warm = consts.tile([P, 1], F32)
    nc.scalar.activation(
        out=exp_warm, in_=pm_tile, func=mybir.ActivationFunctionType.Exp,
        bias=0.0, scale=0.0,
    )

    if reps > 1:
        # Benchmark mode: execute the body `reps` times in one NEFF via a
        # hardware loop so per-rep time is measurable over dispatch overhead.
        with tc.For_i(0, reps, 1, hint_engines=(mybir.EngineType.PE,
                                                mybir.EngineType.DVE,
                                                mybir.EngineType.Activation)):
            _phases(tc, out, x, pm_tile, identity, identity8,
                    v_pool, v8_pool, vt_pool, exp_pool, expt_pool, stat_pool,
                    out_pool, e_psum, t_psum, o_psum)
    else:
        _phases(tc, out, x, pm_tile, identity, identity8,
                v_pool, v8_pool, vt_pool, exp_pool, expt_pool, stat_pool,
                out_pool, e_psum, t_psum, o_psum)


def _phases(tc, out, x, pm_tile, identity, identity8,
            v_pool, v8_pool, vt_pool, exp_pool, expt_pool, stat_pool,
            out_pool, e_psum, t_psum, o_psum):
    nc = tc.nc
    # Load v into ONE stacked f32 tile [P, MT, HW] (channel tile m at free
    # offset m*HW) so each 512-column slab arrives as a single DMA spanning
    # all four channel tiles — 11 input DMAs instead of 32 cuts per-DMA
    # launch overhead off the bus-bound input stream. The last slab is loaded
    # at k-tile (128-col) granularity so the post-input tail chain
    # (cast->transpose->copy->energy->softmax) pipelines at fine grain.
    # Declared f32r with a bitcast DMA (bits unchanged) so the residual-fold
    # f32r matmul sees a sanctioned producer; fp32-exact readers bitcast back.
    Vf = v_pool.tile([P, MT, HW], F32R, name="vf", tag="vf")
    Vx = Vf.bitcast(F32)
    xr = x.rearrange("(m q) p -> q m p", q=P).bitcast(F32R)
    for c in range(7):
        nc.sync.dma_start(out=Vf[:, :, c * LCH:(c + 1) * LCH],
                          in_=xr[:, :, c * LCH:(c + 1) * LCH])
    for k in range(KT - 4, KT):
        nc.sync.dma_start(out=Vf[:, :, k * P:(k + 1) * P],
                          in_=xr[:, :, k * P:(k + 1) * P])

    # Casts f32 -> fp8e4 into the stacked fp8 twin (the layout DoubleRow's
    # paired-k-tile operand slicing needs), split ACT/DVE per (slab, m).
    # Pool/GPSIMD is deliberately idle: it is a software (Q7) engine on this
    # stack and measured far below the other engines on hardware. The final
    # slab casts at k-tile grain on DVE (127 ns each with the all-SBUF 2x
    # mode) so the tail chain pipelines at fine grain.
    V8 = v8_pool.tile([P, MT, HW], F8, name="v8", tag="v8")
    for c in range(7):
        for m in range(MT):
            src = Vx[:, m, c * LCH:(c + 1) * LCH]
            dst = V8[:, m, c * LCH:(c + 1) * LCH]
            if (4 * c + m) % 2 == 0:
                nc.scalar.copy(out=dst, in_=src)
            else:
                nc.vector.tensor_copy(out=dst, in_=src)
    for k in range(KT - 4, KT):
        for m in range(MT):
            nc.vector.tensor_copy(
                out=V8[:, m, k * P:(k + 1) * P],
                in_=Vx[:, m, k * P:(k + 1) * P],
            )

    # Phase 1+2, software-pipelined one double-k-tile ahead: transposes of
    # t+1 run on PE while t's PSUM->SBUF copies are in flight, then t's
    # energy. Walrus requires fp8 transpose outputs at element step 2, so
    # the PSUM staging tile carries a trailing stride-2 dim; the copies
    # compact it, alternating DVE / ACT per k-slab (a single engine can't
    # keep pace with the DMA stream). A uint16-paired variant (half the
    # transposes, 2x DVE copies) simmed 2 us faster but its byte-strided
    # DoubleRow operands hard-crashed the exec unit, so: per-k, stride-2,
    # middle-dim slices only — the exact pattern hardware validated.
    vts = vt_pool.tile([P, KT, C], F8, name="vts", tag="vts")
    E = [e_psum.tile([P, C], F32, name=f"e{m}", tag=f"e{m}") for m in range(MT)]

    def emit_transposes(t):
        tp = t_psum.tile([P, 2, MT, P, 2], F8, tag="tp")
        for i in range(2):
            kk = 2 * t + i
            for m in range(MT):
                nc.tensor.transpose(
                    tp[:, i, m, :, 0], V8[:, m, kk * P:(kk + 1) * P], identity8
                )
            if t >= TT - 2:
                # per-k copies at the tail keep the last-slab chain short
                dst = vts[:, kk, :].rearrange("p (m q) -> p m q", m=MT)
                if kk % 2 == 0:
                    nc.vector.tensor_copy(out=dst, in_=tp[:, i, :, :, 0])
                else:
                    nc.scalar.copy(out=dst, in_=tp[:, i, :, :, 0])
        if t < TT - 2:
            # one paired copy per double tile: fewer, larger PSUM drains
            dst = vts[:, 2 * t:2 * t + 2, :].rearrange(
                "p k (m q) -> p k m q", m=MT
            )
            if t % 2 == 0:
                nc.vector.tensor_copy(out=dst, in_=tp[:, :, :, :, 0])
            else:
                nc.scalar.copy(out=dst, in_=tp[:, :, :, :, 0])

    def emit_energy(t):
        rhs = vts[:, 2 * t:2 * t + 2, :]
        for m in range(MT):
            nc.tensor.matmul(
                E[m],
                lhsT=vts[:, 2 * t:2 * t + 2, m * P:(m + 1) * P],
                rhs=rhs,
                start=(t == 0),
                stop=(t == TT - 1),
                perf_mode=DR,
            )

    for t in range(TT + 1):
        if t < TT:
            emit_transposes(t)
        if t >= 1:
            emit_energy(t - 1)

    # Softmax per row tile: DVE row max straight out of PSUM (Pool/GPSIMD
    # can't reach PSUM, ACT has no max-accumulate), ACT Exp with fp8 output
    # rows and fp32 row-sum accumulation BEFORE the cast (normalization stays
    # exact; the drain scale carries pm/sum). Row tile 0's whole chain is
    # emitted first on every queue; later rows' stats interleave into the
    # phase-4 stream so they never delay row 0's first output bytes.
    EXP = []
    SUMS = []
    SCALE = []
    NEGS = []

    def emit_rm(mi):
        neg_max = stat_pool.tile([P, 1], F32, tag=f"negm{mi}")
        nc.vector.tensor_reduce(
            out=neg_max,
            in_=E[mi],
            op=mybir.AluOpType.max,
            axis=mybir.AxisListType.X,
            negate=True,
        )
        NEGS.append(neg_max)

    def emit_exp(mi):
        exp_t = exp_pool.tile([P, C], F8, name=f"exp{mi}", tag=f"exp{mi}")
        s_t = stat_pool.tile([P, 1], F32, tag=f"s{mi}")
        nc.scalar.activation(
            out=exp_t,
            in_=E[mi],
            func=mybir.ActivationFunctionType.Exp,
            bias=NEGS[mi],
            scale=1.0,
            accum_out=s_t,
        )
        EXP.append(exp_t)
        SUMS.append(s_t)

    # 1/para_mu for the residual-fold diagonal (built once, off the chain)
    recip_pm = stat_pool.tile([P, 1], F32, tag="rpm")
    nc.vector.reciprocal(recip_pm, pm_tile)

    DIAG = []

    def emit_scale(mi):
        rs = stat_pool.tile([P, 1], F32, tag=f"rs{mi}")
        nc.vector.reciprocal(rs, SUMS[mi])
        sc = stat_pool.tile([P, 1], F32, tag=f"sc{mi}")
        nc.vector.tensor_mul(sc, rs, pm_tile)
        SCALE.append(sc)
        # diag(rs/pm) for row tile mi: identity rows scaled per-partition.
        # A third matmul per chunk accumulates diag @ x into PSUM, folding
        # the residual add into PE so the epilogue is a single ACT drain
        # (out = (O + rs/pm * x) * (pm/rs)).
        sd = stat_pool.tile([P, 1], F32, tag=f"sd{mi}")
        nc.vector.tensor_mul(sd, rs, recip_pm)
        dg = stat_pool.tile([P, P], F32, name=f"diag{mi}", tag=f"diag{mi}")
        nc.scalar.mul(dg, identity, sd)
        dg_r = stat_pool.tile([P, P], F32R, name=f"diagr{mi}", tag=f"diagr{mi}")
        nc.vector.tensor_copy(out=dg_r, in_=dg)
        DIAG.append(dg_r)

    # Phase 4: out rows = expT.T @ v in fp8 DoubleRow. Weight loads amortize
    # tt-outer: one [128, 2, 128] stationary load per (mi, half, tt) covers
    # 4 chunks x 512 moving columns. Accumulators rotate over 6 PSUM banks
    # (2 o-banks + the 4 energy banks, dead after the exp pass). Epilogue
    # split four ways: PSUM drains with the pm/sum scale alternate ACT / DVE
    # per chunk, POOL (idle after the casts, full-rate SBUF adds) adds the
    # f32 residual, SP ships DMAs. First and last row tiles ship quarter-row
    # 512 KB pieces so the out stream starts early and ends granularly;
    # middle rows use 1 MB halves.
    EXPT = expt_pool.tile([P, MT, C], F8, name="expt", tag="expt")

    def emit_expt(mi):
        tp2 = t_psum.tile([P, 2, MT, P, 2], F8, tag="tp", name=f"tpx{mi}")
        for mj in range(MT):
            nc.tensor.transpose(
                tp2[:, 0, mj, :, 0], EXP[mi][:, mj * P:(mj + 1) * P], identity8
            )
        nc.vector.tensor_copy(
            out=EXPT[:, :, mi * P:(mi + 1) * P], in_=tp2[:, 0, :, :, 0]
        )

    # rm(mi+1) is emitted early (DVE has an idle slot while ACT runs exp),
    # but exp(mi+1) is deferred to between mi's half-0 and half-1 drain
    # groups on ACT: otherwise every row's first drain waits ~0.8 us behind
    # an exp it doesn't need.
    emit_rm(0)
    emit_exp(0)
    for mi in range(MT):
        if mi + 1 < MT:
            emit_rm(mi + 1)
        emit_scale(mi)
        emit_expt(mi)
        ob = out_pool.tile([P, HW], F32, tag="ob")
        quarters = mi == 0 or mi == MT - 1
        for half in range(2):
            ops = []
            for q4 in range(4):
                ch = half * 4 + q4
                slot = (mi * NCHUNKS + ch) % 6
                if slot < 4:
                    o_ps = e_psum.tile([P, NCH], F32, name=f"ops{slot}", tag=f"e{slot}")
                else:
                    o_ps = o_psum.tile([P, NCH], F32, name=f"ops{slot}", tag="ops")
                ops.append(o_ps)
            for tt in range(2):
                for q4 in range(4):
                    ch = half * 4 + q4
                    nc.tensor.matmul(
                        ops[q4],
                        lhsT=EXPT[:, 2 * tt:2 * tt + 2, mi * P:(mi + 1) * P],
                        rhs=V8[:, 2 * tt:2 * tt + 2, ch * NCH:(ch + 1) * NCH],
                        start=(tt == 0),
                        stop=(tt == 1),
                        perf_mode=DR,
                    )
            for q4 in range(4):
                # residual fold: += diag(rs/pm) @ x, f32r at 1 cyc/row
                ch = half * 4 + q4
                nc.tensor.matmul(
                    ops[q4],
                    lhsT=DIAG[mi],
                    rhs=Vf[:, mi, ch * NCH:(ch + 1) * NCH],
                    start=False,
                    stop=True,
                    skip_group_check=True,
                )
            for q4 in range(4):
                ch = half * 4 + q4
                obc = ob[:, ch * NCH:(ch + 1) * NCH]
                if q4 % 2 == 0:
                    nc.scalar.mul(obc, ops[q4], SCALE[mi])
                else:
                    nc.vector.tensor_scalar_mul(obc, ops[q4], SCALE[mi])
                first_piece = mi == 0 and half == 0 and q4 < 2
                if first_piece:
                    # single-chunk pieces: the first output bytes gate the
                    # whole out stream, ship them the moment chunk 0 lands
                    nc.sync.dma_start(
                        out=out[mi * P:(mi + 1) * P, ch * NCH:(ch + 1) * NCH],
                        in_=ob[:, ch * NCH:(ch + 1) * NCH],
                    )
                elif quarters and q4 % 2 == 1:
                    nc.sync.dma_start(
                        out=out[mi * P:(mi + 1) * P, (ch - 1) * NCH:(ch + 1) * NCH],
                        in_=ob[:, (ch - 1) * NCH:(ch + 1) * NCH],
                    )
            if not quarters:
                nc.sync.dma_start(
                    out=out[mi * P:(mi + 1) * P, half * (HW // 2):(half + 1) * (HW // 2)],
                    in_=ob[:, half * (HW // 2):(half + 1) * (HW // 2)],
                )
            if half == 0 and mi + 1 < MT:
                emit_exp(mi + 1)


def build_nc(reps: int = 1) -> bass.Bass:
    # bacc.Bacc (not raw bass.Bass): its compile() pass legalizes multi-sem
    # waits into explicit event-semaphore instructions (walrus allows only one
    # sync wait per TPB instruction).
    nc = bacc.Bacc("TRN2", debug=False)
    x = nc.dram_tensor("x", [C, HW], F32, kind="ExternalInput").ap()
    pm = nc.dram_tensor("para_mu", [1], F32, kind="ExternalInput").ap()
    out = nc.dram_tensor("out", [C, HW], F32, kind="ExternalOutput").ap()
    with tile.TileContext(nc) as tc, ExitStack() as ctx:
        _body(ctx, tc, out, x, pm, reps=reps)
    nc.compile()
    return nc


_nc_cache = None


def run(x: np.ndarray, para_mu: np.ndarray, **spmd_kwargs):
    """Run on 8 NeuronCores; returns (output [8,512,64,64], BassKernelResults)."""
    global _nc_cache
    x = np.ascontiguousarray(np.asarray(x, dtype=np.float32))
    pm = np.ascontiguousarray(np.asarray(para_mu, dtype=np.float32).reshape(1))
    assert x.shape == (N, C, H, W), x.shape
    if _nc_cache is None:
        _nc_cache = build_nc()
    in_maps = [
        {"x": x[n].reshape(C, HW), "para_mu": pm} for n in range(N)
    ]
    res = run_bass_kernel_spmd(_nc_cache, in_maps, core_ids=list(range(N)), **spmd_kwargs)
    out = np.stack(
        [np.asarray(res.results[n]["out"]).reshape(C, H, W) for n in range(N)]
    )
    return out, res


def kernel(x: np.ndarray, para_mu: np.ndarray) -> np.ndarray:
    out, _ = run(x, para_mu)
    return out


# revision 34
# speedup vs baseline: 1.0081x; 1.0081x over previous
"""Trainium2 Bass kernel for nn_CAM (channel attention module), fp8 edition.

Reference computation (per batch element n):
    v = x[n].reshape(C, H*W)                      # [512, 4096]
    energy = v @ v.T                              # [512, 512]
    attn = softmax(energy, axis=-1)
    out = attn @ v                                # [512, 4096]
    result = para_mu * out + x[n]

Sharding: data-parallel over batch N=8, one batch element per NeuronCore
(8 cores). Everything is core-local — no collectives.

Numerics: for unit-gaussian x, energy's diagonal (||v_i||^2 ~ 4096) towers
~3500 above the off-diagonal (max ~450), so softmax is a hair from one-hot
and fp8 quantization of v / exp leaves the result well inside tolerance
(measured 3.7e-3 max-rel vs fp64 for the actual inputs; fp32 baseline was
1.8e-4 against a 2e-2 gate).

Kernel strategy (per core):
  1. Chunked DMA of v (natural layout, C on partitions) into SBUF f32 as 4
     [128, 4096] tiles, column-slab interleaved so compute starts early.
  2. Pool engine casts each slab to fp8e4 into V8 [128, MT, HW] (stacked
     channel tiles share partitions — the layout DoubleRow's paired-k-tile
     operand slicing needs). Pool is otherwise idle; ACT/DVE stay free.
  3. PE transposes fp8 v-slabs (1.0 cyc/row vs 1.5 f32r) into vts
     [128, KT, C]; one DVE copy per k-slab moves PSUM->SBUF.
  4. Energy in fp8e4 DoubleRow (0.5 cyc/row, K=256 per instruction): per
     double-k-tile t, E[m] += vts[:, 2t:2t+2, m-block].T (x) vts[:, 2t:2t+2, :].
     Full 512-wide rows (no symmetry trick): softmax then reads E straight
     from PSUM — no SBUF reconstruction pass on the critical path.
  5. Row softmax: DVE reduce_max (negated) from PSUM -> ACT Exp with
     per-partition bias, fp32 accumulated row sum, fp8 output rows; DVE
     reciprocal; scale = para_mu/rowsum carried to the epilogue. The Exp
     activation table is preloaded during the load phase (1.3 us off the
     post-barrier chain).
  6. expT via fp8 PE transposes (one row tile ahead of its matmuls);
     out_unnorm = expT.T @ V8 in DoubleRow, tt-outer per half-row so each
     [128, 2, 128] weight load amortizes over 4 x 512 moving columns.
     Accumulators rotate over 6 PSUM banks (2 o-banks + 4 energy banks,
     dead after the exp pass).
  7. Epilogue split across engines: ACT scales out of PSUM by pm/rowsum,
     DVE adds the f32 residual x at SBUF 2x rate; 1 MB half-row DMAs, the
     final half in two 512 KB pieces to shorten the kernel tail.
"""

import sys

if "/opt/trn_rl_repo" not in sys.path:
    sys.path.insert(0, "/opt/trn_rl_repo")

from contextlib import ExitStack

import numpy as np

import concourse.bass as bass
import concourse.mybir as mybir
import concourse.tile as tile
from concourse import bacc
from concourse.bass_utils import run_bass_kernel_spmd
from concourse.masks import make_identity

N, C, H, W = 8, 512, 64, 64
HW = H * W            # 4096
P = 128               # partitions
MT = C // P           # 4 row tiles of the channel dim
KT = HW // P          # 32 contraction k-tiles for the energy matmul
TT = KT // 2          # 16 DoubleRow double-k-tiles
NCH = 512             # free-dim chunk for the output matmul (one PSUM bank)
NCHUNKS = HW // NCH   # 8
LCH = HW // 8         # 512-column DMA load slabs
F32 = mybir.dt.float32
F32R = mybir.dt.float32r
F8 = mybir.dt.float8e4
U16 = mybir.dt.uint16
DR = mybir.MatmulPerfMode.DoubleRow


def _body(ctx: ExitStack, tc: "tile.TileContext", out: bass.AP, x: bass.AP, pm: bass.AP,
          reps: int = 1):
    nc = tc.nc
    consts = ctx.enter_context(tc.tile_pool(name="consts", bufs=1))
    v_pool = ctx.enter_context(tc.tile_pool(name="v", bufs=1))
    v8_pool = ctx.enter_context(tc.tile_pool(name="v8", bufs=1))
    vt_pool = ctx.enter_context(tc.tile_pool(name="vt", bufs=1))
    exp_pool = ctx.enter_context(tc.tile_pool(name="exp", bufs=1))
    expt_pool = ctx.enter_context(tc.tile_pool(name="expt", bufs=1))
    stat_pool = ctx.enter_context(tc.tile_pool(name="stats", bufs=1))
    out_pool = ctx.enter_context(tc.tile_pool(name="ob", bufs=2))
    e_psum = ctx.enter_context(tc.tile_pool(name="e_ps", bufs=1, space="PSUM"))
    t_psum = ctx.enter_context(tc.tile_pool(name="t_ps", bufs=2, space="PSUM"))
    o_psum = ctx.enter_context(tc.tile_pool(name="o_ps", bufs=2, space="PSUM"))

    identity = consts.tile([P, P], F32)
    nc.vector.memset(identity, 0.0)
    make_identity(nc, identity, nomemset=True)
    # fp8 twin: transposes of fp8 data need an fp8 permutation ifmap (and run
    # at 1.0 cyc/row). 0.0/1.0 are exact in fp8e4.
    identity8 = consts.tile([P, P], F8)
    nc.vector.tensor_copy(out=identity8, in_=identity)

    # emitted after make_identity: the gpsimd queue is serial, and this DMA
    # ahead of affine_select would delay the Pool cast stream
    pm_tile = consts.tile([P, 1], F32)
    nc.gpsimd.dma_start(out=pm_tile, in_=pm.to_broadcast((P, 1)))
    # Preload the Exp activation table (1283 ns) while DMAs stream so the
    # post-barrier softmax doesn't eat the table switch.
    exp_warm = consts.tile([P, 1], F32)
    nc.scalar.activation(
        out=exp_warm, in_=pm_tile, func=mybir.ActivationFunctionType.Exp,
        bias=0.0, scale=0.0,
    )

    if reps > 1:
        # Benchmark mode: execute the body `reps` times in one NEFF via a
        # hardware loop so per-rep time is measurable over dispatch overhead.
        with tc.For_i(0, reps, 1, hint_engines=(mybir.EngineType.PE,
                                                mybir.EngineType.DVE,
                                                mybir.EngineType.Activation)):
            _phases(tc, out, x, pm_tile, identity, identity8,
                    v_pool, v8_pool, vt_pool, exp_pool, expt_pool, stat_pool,
                    out_pool, e_psum, t_psum, o_psum)
    else:
        _phases(tc, out, x, pm_tile, identity, identity8,
                v_pool, v8_pool, vt_pool, exp_pool, expt_pool, stat_pool,
                out_pool, e_psum, t_psum, o_psum)


def _phases(tc, out, x, pm_tile, identity, identity8,
            v_pool, v8_pool, vt_pool, exp_pool, expt_pool, stat_pool,
            out_pool, e_psum, t_psum, o_psum):
    nc = tc.nc
    # Load v into ONE stacked f32 tile [P, MT, HW] (channel tile m at free
    # offset m*HW) so each 512-column slab arrives as a single DMA spanning
    # all four channel tiles — 11 input DMAs instead of 32 cuts per-DMA
    # launch overhead off the bus-bound input stream. The last slab is loaded
    # at k-tile (128-col) granularity so the post-input tail chain
    # (cast->transpose->copy->energy->softmax) pipelines at fine grain.
    # Declared f32r with a bitcast DMA (bits unchanged) so the residual-fold
    # f32r matmul sees a sanctioned producer; fp32-exact readers bitcast back.
    Vf = v_pool.tile([P, MT, HW], F32R, name="vf", tag="vf")
    Vx = Vf.bitcast(F32)
    xr = x.rearrange("(m q) p -> q m p", q=P).bitcast(F32R)
    for c in range(7):
        nc.sync.dma_start(out=Vf[:, :, c * LCH:(c + 1) * LCH],
                          in_=xr[:, :, c * LCH:(c + 1) * LCH])
    for k in range(KT - 4, KT):
        nc.sync.dma_start(out=Vf[:, :, k * P:(k + 1) * P],
                          in_=xr[:, :, k * P:(k + 1) * P])

    # Casts f32 -> fp8e4 into the stacked fp8 twin (the layout DoubleRow's
    # paired-k-tile operand slicing needs), split ACT/DVE per (slab, m).
    # Pool/GPSIMD is deliberately idle: it is a software (Q7) engine on this
    # stack and measured far below the other engines on hardware. The final
    # slab casts at k-tile grain on DVE (127 ns each with the all-SBUF 2x
    # mode) so the tail chain pipelines at fine grain.
    V8 = v8_pool.tile([P, MT, HW], F8, name="v8", tag="v8")
    for c in range(7):
        for m in range(MT):
            src = Vx[:, m, c * LCH:(c + 1) * LCH]
            dst = V8[:, m, c * LCH:(c + 1) * LCH]
            if (4 * c + m) % 2 == 0:
                nc.scalar.copy(out=dst, in_=src)
            else:
                nc.vector.tensor_copy(out=dst, in_=src)
    for k in range(KT - 4, KT):
        for m in range(MT):
            nc.vector.tensor_copy(
                out=V8[:, m, k * P:(k + 1) * P],
                in_=Vx[:, m, k * P:(k + 1) * P],
            )

    # Phase 1+2, software-pipelined one double-k-tile ahead: transposes of
    # t+1 run on PE while t's PSUM->SBUF copies are in flight, then t's
    # energy. Walrus requires fp8 transpose outputs at element step 2, so
    # the PSUM staging tile carries a trailing stride-2 dim; the copies
    # compact it, alternating DVE / ACT per k-slab (a single engine can't
    # keep pace with the DMA stream). A uint16-paired variant (half the
    # transposes, 2x DVE copies) simmed 2 us faster but its byte-strided
    # DoubleRow operands hard-crashed the exec unit, so: per-k, stride-2,
    # middle-dim slices only — the exact pattern hardware validated.
    vts = vt_pool.tile([P, KT, C], F8, name="vts", tag="vts")
    E = [e_psum.tile([P, C], F32, name=f"e{m}", tag=f"e{m}") for m in range(MT)]

    def emit_transposes(t):
        tp = t_psum.tile([P, 2, MT, P, 2], F8, tag="tp")
        for i in range(2):
            kk = 2 * t + i
            for m in range(MT):
                nc.tensor.transpose(
                    tp[:, i, m, :, 0], V8[:, m, kk * P:(kk + 1) * P], identity8
                )
            if t >= TT - 2:
                # per-k copies at the tail keep the last-slab chain short
                dst = vts[:, kk, :].rearrange("p (m q) -> p m q", m=MT)
                if kk % 2 == 0:
                    nc.vector.tensor_copy(out=dst, in_=tp[:, i, :, :, 0])
                else:
                    nc.scalar.copy(out=dst, in_=tp[:, i, :, :, 0])
        if t < TT - 2:
            # one paired copy per double tile: fewer, larger PSUM drains
            dst = vts[:, 2 * t:2 * t + 2, :].rearrange(
                "p k (m q) -> p k m q", m=MT
            )
            if t % 2 == 0:
                nc.vector.tensor_copy(out=dst, in_=tp[:, :, :, :, 0])
            else:
                nc.scalar.copy(out=dst, in_=tp[:, :, :, :, 0])

    def emit_energy(t):
        rhs = vts[:, 2 * t:2 * t + 2, :]
        for m in range(MT):
            nc.tensor.matmul(
                E[m],
                lhsT=vts[:, 2 * t:2 * t + 2, m * P:(m + 1) * P],
                rhs=rhs,
                start=(t == 0),
                stop=(t == TT - 1),
                perf_mode=DR,
            )

    for t in range(TT + 1):
        if t < TT:
            emit_transposes(t)
        if t >= 1:
            emit_energy(t - 1)

    # Softmax per row tile: DVE row max straight out of PSUM (Pool/GPSIMD
    # can't reach PSUM, ACT has no max-accumulate), ACT Exp with fp8 output
    # rows and fp32 row-sum accumulation BEFORE the cast (normalization stays
    # exact; the drain scale carries pm/sum). Row tile 0's whole chain is
    # emitted first on every queue; later rows' stats interleave into the
    # phase-4 stream so they never delay row 0's first output bytes.
    EXP = []
    SUMS = []
    SCALE = []

    def emit_stats(mi):
        neg_max = stat_pool.tile([P, 1], F32, tag=f"negm{mi}")
        nc.vector.tensor_reduce(
            out=neg_max,
            in_=E[mi],
            op=mybir.AluOpType.max,
            axis=mybir.AxisListType.X,
            negate=True,
        )
        exp_t = exp_pool.tile([P, C], F8, name=f"exp{mi}", tag=f"exp{mi}")
        s_t = stat_pool.tile([P, 1], F32, tag=f"s{mi}")
        nc.scalar.activation(
            out=exp_t,
            in_=E[mi],
            func=mybir.ActivationFunctionType.Exp,
            bias=neg_max,
            scale=1.0,
            accum_out=s_t,
        )
        EXP.append(exp_t)
        SUMS.append(s_t)

    # 1/para_mu for the residual-fold diagonal (built once, off the chain)
    recip_pm = stat_pool.tile([P, 1], F32, tag="rpm")
    nc.vector.reciprocal(recip_pm, pm_tile)

    DIAG = []

    def emit_scale(mi):
        rs = stat_pool.tile([P, 1], F32, tag=f"rs{mi}")
        nc.vector.reciprocal(rs, SUMS[mi])
        sc = stat_pool.tile([P, 1], F32, tag=f"sc{mi}")
        nc.vector.tensor_mul(sc, rs, pm_tile)
        SCALE.append(sc)
        # diag(rs/pm) for row tile mi: identity rows scaled per-partition.
        # A third matmul per chunk accumulates diag @ x into PSUM, folding
        # the residual add into PE so the epilogue is a single ACT drain
        # (out = (O + rs/pm * x) * (pm/rs)).
        sd = stat_pool.tile([P, 1], F32, tag=f"sd{mi}")
        nc.vector.tensor_mul(sd, rs, recip_pm)
        dg = stat_pool.tile([P, P], F32, name=f"diag{mi}", tag=f"diag{mi}")
        nc.scalar.mul(dg, identity, sd)
        dg_r = stat_pool.tile([P, P], F32R, name=f"diagr{mi}", tag=f"diagr{mi}")
        nc.vector.tensor_copy(out=dg_r, in_=dg)
        DIAG.append(dg_r)

    # Phase 4: out rows = expT.T @ v in fp8 DoubleRow. Weight loads amortize
    # tt-outer: one [128, 2, 128] stationary load per (mi, half, tt) covers
    # 4 chunks x 512 moving columns. Accumulators rotate over 6 PSUM banks
    # (2 o-banks + the 4 energy banks, dead after the exp pass). Epilogue
    # split four ways: PSUM drains with the pm/sum scale alternate ACT / DVE
    # per chunk, POOL (idle after the casts, full-rate SBUF adds) adds the
    # f32 residual, SP ships DMAs. First and last row tiles ship quarter-row
    # 512 KB pieces so the out stream starts early and ends granularly;
    # middle rows use 1 MB halves.
    EXPT = expt_pool.tile([P, MT, C], F8, name="expt", tag="expt")

    def emit_expt(mi):
        tp2 = t_psum.tile([P, 2, MT, P, 2], F8, tag="tp", name=f"tpx{mi}")
        for mj in range(MT):
            nc.tensor.transpose(
                tp2[:, 0, mj, :, 0], EXP[mi][:, mj * P:(mj + 1) * P], identity8
            )
        nc.vector.tensor_copy(
            out=EXPT[:, :, mi * P:(mi + 1) * P], in_=tp2[:, 0, :, :, 0]
        )

    emit_stats(0)
    for mi in range(MT):
        if mi + 1 < MT:
            emit_stats(mi + 1)
        emit_scale(mi)
        emit_expt(mi)
        ob = out_pool.tile([P, HW], F32, tag="ob")
        quarters = mi == 0 or mi == MT - 1
        for half in range(2):
            ops = []
            for q4 in range(4):
                ch = half * 4 + q4
                slot = (mi * NCHUNKS + ch) % 6
                if slot < 4:
                    o_ps = e_psum.tile([P, NCH], F32, name=f"ops{slot}", tag=f"e{slot}")
                else:
                    o_ps = o_psum.tile([P, NCH], F32, name=f"ops{slot}", tag="ops")
                ops.append(o_ps)
            for tt in range(2):
                for q4 in range(4):
                    ch = half * 4 + q4
                    nc.tensor.matmul(
                        ops[q4],
                        lhsT=EXPT[:, 2 * tt:2 * tt + 2, mi * P:(mi + 1) * P],
                        rhs=V8[:, 2 * tt:2 * tt + 2, ch * NCH:(ch + 1) * NCH],
                        start=(tt == 0),
                        stop=(tt == 1),
                        perf_mode=DR,
                    )
            for q4 in range(4):
                # residual fold: += diag(rs/pm) @ x, f32r at 1 cyc/row
                ch = half * 4 + q4
                nc.tensor.matmul(
                    ops[q4],
                    lhsT=DIAG[mi],
                    rhs=Vf[:, mi, ch * NCH:(ch + 1) * NCH],
                    start=False,
                    stop=True,
                    skip_group_check=True,
                )
            for q4 in range(4):
                ch = half * 4 + q4
                obc = ob[:, ch * NCH:(ch + 1) * NCH]
                if q4 % 2 == 0:
                    nc.scalar.mul(obc, ops[q4], SCALE[mi])
                else:
                    nc.vector.tensor_scalar_mul(obc, ops[q4], SCALE[mi])
                first_piece = mi == 0 and half == 0 and q4 < 2
                if first_piece:
                    # single-chunk pieces: the first output bytes gate the
                    # whole out stream, ship them the moment chunk 0 lands
                    nc.sync.dma_start(
                        out=out[mi * P:(mi + 1) * P, ch * NCH:(ch + 1) * NCH],
                        in_=ob[:, ch * NCH:(ch + 1) * NCH],
                    )
                elif quarters and q4 % 2 == 1:
                    nc.sync.dma_start(
                        out=out[mi * P:(mi + 1) * P, (ch - 1) * NCH:(ch + 1) * NCH],
                        in_=ob[:, (ch - 1) * NCH:(ch + 1) * NCH],
                    )
            if not quarters:
                nc.sync.dma_start(
                    out=out[mi * P:(mi + 1) * P, half * (HW // 2):(half + 1) * (HW // 2)],
                    in_=ob[:, half * (HW // 2):(half + 1) * (HW // 2)],
                )


def build_nc(reps: int = 1) -> bass.Bass:
    # bacc.Bacc (not raw bass.Bass): its compile() pass legalizes multi-sem
    # waits into explicit event-semaphore instructions (walrus allows only one
    # sync wait per TPB instruction).
    nc = bacc.Bacc("TRN2", debug=False)
    x = nc.dram_tensor("x", [C, HW], F32, kind="ExternalInput").ap()
    pm = nc.dram_tensor("para_mu", [1], F32, kind="ExternalInput").ap()
    out = nc.dram_tensor("out", [C, HW], F32, kind="ExternalOutput").ap()
    with tile.TileContext(nc) as tc, ExitStack() as ctx:
        _body(ctx, tc, out, x, pm, reps=reps)
    nc.compile()
    return nc


_nc_cache = None


def run(x: np.ndarray, para_mu: np.ndarray, **spmd_kwargs):
    """Run on 8 NeuronCores; returns (output [8,512,64,64], BassKernelResults)."""
    global _nc_cache
    x = np.ascontiguousarray(np.asarray(x, dtype=np.float32))
    pm = np.ascontiguousarray(np.asarray(para_mu, dtype=np.float32).reshape(1))
    assert x.shape == (N, C, H, W), x.shape
    if _nc_cache is None:
        _nc_cache = build_nc()
    in_maps = [
        {"x": x[n].reshape(C, HW), "para_mu": pm} for n in range(N)
    ]
    res = run_bass_kernel_spmd(_nc_cache, in_maps, core_ids=list(range(N)), **spmd_kwargs)
    out = np.stack(
        [np.asarray(res.results[n]["out"]).reshape(C, H, W) for n in range(N)]
    )
    return out, res


def kernel(x: np.ndarray, para_mu: np.ndarray) -> np.ndarray:
    out, _ = run(x, para_mu)
    return out
